# revision 32
# baseline (speedup 1.0000x reference)
"""Compact Bilinear Pooling on 8 Trainium2 NeuronCores.

Math: for each sample b, Output[b] = sum over pixels p of
  countsketch(x1_p) (circular-conv) countsketch(x2_p)
which, because the sum over pixels commutes with the bilinear pair
products, equals a scatter-reduce of the per-sample gram matrix
  G_b[c1, c2] = sum_p x1[b,p,c1] * x2[b,p,c2]
into buckets d = (h1[c1] + h2[c2]) mod 8192 with signs s1[c1]*s2[c2].

Device plan (two launches, both index-independent programs):
  Phase 1 (batch-sharded, 4 samples/core): G_b = X1_b^T @ X2_b on the
    tensor engine -> DRAM (bf16). Pixels are pre-split into two K=98
    halves so each sample needs exactly one load DMA per input tensor
    (the phase is DMA-bound; HWDGE descriptor-gen serializes per DMA
    instruction, so few big DMAs win).
  Host: zero-FLOP reshard. Pair -> (bucket, slot) is compile-time data.
    Buckets are sharded 1024/core and sorted by occupancy so that each
    128-bucket block needs only (max count in block) slot-levels; the
    padded bucket-major table is therefore near-minimal (~2.2MB/core
    instead of 4MB with a global-max cap). Sketch signs are folded in
    as a sign-bit flip (part of the hash, no FLOPs).
  Phase 2 (bucket-sharded): segmented sums on the TENSOR engine:
    for each slot-level s of a block, matmul with a 128x128 identity
    accumulates table[:, s-level] into the block's PSUM region
    (psum[r%128, b] += t[r%128, (blk,s), b]). PSUM accumulates in f32,
    one copy + one store emit the result.
"""

import numpy as np
import ml_dtypes

import concourse.bass as bass
import concourse.bacc as bacc
import concourse.mybir as mybir
from concourse.tile import TileContext
from concourse import bass_utils

B, C, HW, D = 32, 512, 196, 8192
NCORES = 8
BPC = B // NCORES          # samples per core in phase 1
DPC = D // NCORES          # buckets per core in phase 2
NBLK = DPC // 128          # 128-bucket blocks per core in phase 2
KH = HW // 2               # pixel (contraction) half: 98 <= 128 partitions
F32 = mybir.dt.float32
F32R = mybir.dt.float32r   # TF32-like PE mode: 1 cycle/row vs 4 for fp32
BF16 = mybir.dt.bfloat16
BF16_NP = ml_dtypes.bfloat16

_cache = {}
_last_runs = []  # (nc, in_maps) of the most recent kernel() call, for profiling


def _build_phase1():
    """Per core: x1,x2 [98, BPC, 2, 512] f32 (pixel-half-major) ->
    g [128, BPC, 4, 512] bf16 where G[b, 128m+p, c2] = g[p, b, m, c2]."""
    nc = bacc.Bacc("TRN2", target_bir_lowering=False, debug=False,
                   num_devices=NCORES)
    x1 = nc.dram_tensor("x1", [KH, BPC, 2, C], F32R, kind="ExternalInput").ap()
    x2 = nc.dram_tensor("x2", [KH, BPC, 2, C], F32R, kind="ExternalInput").ap()
    g = nc.dram_tensor("g", [128, BPC, 4, C], BF16, kind="ExternalOutput").ap()

    with TileContext(nc) as tc:
        with (
            tc.tile_pool(name="xp", bufs=3) as xp,
            tc.tile_pool(name="gp", bufs=2) as gp,
            tc.tile_pool(name="ps", bufs=1, space="PSUM") as ps,
        ):
            # DMA issue rotates over the three DMA-capable engines (SP /
            # Activation / Pool HWDGE+SWDGE queues run in parallel); loads
            # are half-sample (one pixel chunk) so the first matmul only
            # waits for two small parallel transfers. psum->sbuf downcast
            # copies go to DVE (otherwise idle) and Pool.
            # Pool/GPSIMD cannot touch PSUM on real hardware, so the
            # psum->sbuf downcast copies go to DVE (most) and Activation
            # (the rest); Pool contributes DMA bandwidth instead.
            di = 0
            # tiny warm-up matmuls while the first loads are in flight: the
            # cost model ramps the PE clock with sustained use (full speed
            # after 3us busy), so idling the PE during the load latency
            # would leave the first real matmuls at the slow p-state.
            ws = xp.tile([KH, 128], BF16, tag="warm")
            nc.vector.memset(ws[:], 0)
            # preload the activation table so Act's first real copy is cheap
            wa = gp.tile([1, 8], BF16, tag="wact")
            nc.scalar.copy(wa[:], ws[0:1, 0:8])
            wp = ps.tile([128, C], F32, name="wpt", tag="pt0_0")
            for w in range(22):
                nc.tensor.matmul(wp[0:64, 0:128], ws[:, 0:64], ws[:],
                                 start=True, stop=True)
            # explicit DMA engine schedule: SP and Pool carry the early
            # loads (Act is busy with its table preload + copies); stores
            # spread so the final ones land on engines idle by then.
            ldeng = [nc.sync, nc.gpsimd, nc.sync, nc.gpsimd,
                     nc.sync, nc.gpsimd, nc.scalar, nc.sync,
                     nc.gpsimd, nc.scalar, nc.sync, nc.gpsimd,
                     nc.scalar, nc.sync, nc.gpsimd, nc.scalar]
            steng = [nc.sync, nc.gpsimd, nc.gpsimd, nc.sync,
                     nc.sync, nc.gpsimd]
            cpeng = [nc.vector.tensor_copy, nc.vector.tensor_copy,
                     nc.vector.tensor_copy, nc.scalar.copy]
            li = si = 0
            for b in range(BPC):
                xa = xp.tile([KH, 2, C], F32R, tag="x1")
                xb = xp.tile([KH, 2, C], F32R, tag="x2")
                for c in range(2):
                    ldeng[li].dma_start(xa[:, c], x1[:, b, c]); li += 1
                    ldeng[li].dma_start(xb[:, c], x2[:, b, c]); li += 1
                gt = gp.tile([128, 4, C], BF16, tag="g")
                pts = [ps.tile([128, C], F32, name=f"pt{m}",
                               tag=f"pt{m}_{b % 2}") for m in range(4)]
                for c in range(2):
                    for m in range(4):
                        nc.tensor.matmul(pts[m][:],
                                         xa[:, c, m * 128:(m + 1) * 128],
                                         xb[:, c, :],
                                         start=(c == 0), stop=(c == 1))
                if b < BPC - 1:
                    for m in range(4):
                        cpeng[m](gt[:, m, :], pts[m][:])
                    for h in range(2):
                        steng[si].dma_start(g[:, b, 2 * h:2 * h + 2],
                                            gt[:, 2 * h:2 * h + 2]); si += 1
                else:
                    # last sample: per-m copies/stores on engines that are
                    # idle by now, so the drain chain after the final matmul
                    # is as short as possible
                    last_cp = [nc.vector.tensor_copy, nc.scalar.copy,
                               nc.vector.tensor_copy, nc.scalar.copy]
                    last_st = [nc.sync, nc.gpsimd, nc.sync, nc.gpsimd]
                    for m in range(4):
                        last_cp[m](gt[:, m, :], pts[m][:])
                        last_st[m].dma_start(g[:, b, m:m + 1],
                                             gt[:, m:m + 1])
    nc.compile()
    return nc


def _build_phase2(S):
    """Per core: t [128, sum(S), 32] bf16 (bucket-rank-major packed pair
    values: partition = rank%128, col group = (block, slot-level), lane =
    sample), ident [128, 128] bf16 -> out [128, NBLK, 32] f32 with
    out[p, blk, b] = sum over slot-levels of t[p, (blk, s), b]."""
    nc = bacc.Bacc("TRN2", target_bir_lowering=False, debug=False,
                   num_devices=NCORES)
    tot = sum(S)
    ident = nc.dram_tensor("ident", [128, 128], BF16,
                           kind="ExternalInput").ap()
    t = nc.dram_tensor("t", [128, tot, B], BF16, kind="ExternalInput").ap()
    out = nc.dram_tensor("out", [128, NBLK, B], F32,
                         kind="ExternalOutput").ap()
    smax = max(S)

    with TileContext(nc) as tc:
        with (
            tc.tile_pool(name="wp", bufs=1) as wp,
            tc.tile_pool(name="tp", bufs=1) as tp,
            tc.tile_pool(name="op", bufs=1) as op,
            tc.tile_pool(name="ps", bufs=4, space="PSUM") as ps,
        ):
            it = wp.tile([128, 128], BF16, tag="ident")
            nc.sync.dma_start(it[:], ident)
            ot = op.tile([128, NBLK, B], F32, tag="ot")
            dmae = [nc.scalar, nc.gpsimd, nc.sync]
            # PE clock warm-up while the first table pieces are in flight
            wt = wp.tile([128, 128], BF16, tag="warmt")
            nc.vector.memset(wt[:], 0)
            wpp = ps.tile([128, 128], F32, name="wpp", tag="warmp")
            for w in range(18):
                nc.tensor.matmul(wpp[0:64, :], wt[0:64, 0:64], wt[0:64, :],
                                 start=True, stop=True)
            # All table loads are issued up front (every block resident in
            # SBUF) and rotate across the three parallel DMA engines. Block 0
            # is split into two tiles so its first matmuls only wait for one
            # small transfer. Blocks 1..4 get their even/odd slot-levels
            # pre-folded on the (otherwise idle) DVE at bf16 2x rate, halving
            # the tensor-engine work for those blocks.
            FOLD = set(range(4, NBLK))
            h0 = 24
            pieces = []          # per blk: list of (tile, start_slot, nslots)
            di = 0
            col = 0
            for blk in range(NBLK):
                s_blk = S[blk]
                if blk == 0:
                    ta = tp.tile([128, h0, B], BF16, tag="t0a")
                    tb = tp.tile([128, smax - h0, B], BF16, tag="t0b")
                    dmae[di % 3].dma_start(ta[:], t[:, col:col + h0, :])
                    di += 1
                    dmae[di % 3].dma_start(tb[:, 0:s_blk - h0, :],
                                           t[:, col + h0:col + s_blk, :])
                    di += 1
                    pieces.append([(ta, 0, h0), (tb, h0, s_blk - h0)])
                else:
                    tt = tp.tile([128, smax, B], BF16, tag=f"t{blk}")
                    dmae[di % 3].dma_start(tt[:, 0:s_blk, :],
                                           t[:, col:col + s_blk, :])
                    di += 1
                    pieces.append([(tt, 0, s_blk)])
                col += s_blk
            folded = {}
            for blk in sorted(FOLD):
                (tt, _, s_blk), = pieces[blk]
                f = s_blk // 2
                ft = tp.tile([128, smax // 2, B], BF16, name="ft",
                             tag=f"f{blk}")
                nc.vector.tensor_tensor(ft[:, 0:f, :], tt[:, 0:2 * f:2, :],
                                        tt[:, 1:2 * f:2, :],
                                        op=mybir.AluOpType.add)
                folded[blk] = (ft, f, s_blk - 2 * f)
            for blk in range(NBLK):
                s_blk = S[blk]
                pt = ps.tile([128, B], F32)
                if blk in folded:
                    ft, f, rest = folded[blk]
                    (tt, _, _), = pieces[blk]
                    nmm = f + rest
                    for s in range(f):
                        nc.tensor.matmul(pt[:], it[:], ft[:, s, :],
                                         start=(s == 0), stop=(s == nmm - 1))
                    for j in range(rest):
                        s = f + j
                        nc.tensor.matmul(pt[:], it[:], tt[:, 2 * f + j, :],
                                         start=(s == 0), stop=(s == nmm - 1))
                else:
                    s = 0
                    for tt, _, ns in pieces[blk]:
                        for j in range(ns):
                            nc.tensor.matmul(pt[:], it[:], tt[:, j, :],
                                             start=(s == 0),
                                             stop=(s == s_blk - 1))
                            s += 1
                nc.vector.tensor_copy(ot[:, blk, :], pt[:])
                if blk == NBLK - 2:
                    nc.scalar.dma_start(out[:, 0:NBLK - 1, :],
                                        ot[:, 0:NBLK - 1, :])
            nc.sync.dma_start(out[:, NBLK - 1:NBLK, :],
                              ot[:, NBLK - 1:NBLK, :])
    nc.compile()
    return nc


def _run(nc, in_maps):
    _last_runs.append((nc, in_maps))
    res = bass_utils.run_bass_kernel_spmd(nc, in_maps,
                                          core_ids=list(range(NCORES)))
    return res.results


def _plan_tables(rand_h1, rand_s1, rand_h2, rand_s2):
    """Pure index bookkeeping (no float math on data): for every (c1, c2)
    pair its bucket d = (h1+h2) % D and sign; buckets sharded DPC/core and
    sorted by occupancy (rank) so 128-bucket blocks are uniformly packed."""
    h1 = rand_h1.astype(np.int64)
    h2 = rand_h2.astype(np.int64)
    bucket = ((h1[:, None] + h2[None, :]) % D).ravel()
    # sign = (2 s1 - 1)(2 s2 - 1) = +1 iff s1 == s2
    pos = (rand_s1[:, None] == rand_s2[None, :]).ravel()

    cnt = np.bincount(bucket, minlength=D)
    # per-core occupancy sort (descending, stable): rank within core
    rank = np.empty(D, np.int64)   # bucket -> rank (0..DPC-1 within core)
    unrank = np.empty((NCORES, DPC), np.int64)  # core, rank -> bucket
    for k in range(NCORES):
        lo = k * DPC
        order = np.argsort(-cnt[lo:lo + DPC], kind="stable")
        unrank[k] = lo + order
        rank[lo + order] = np.arange(DPC)

    # global (SPMD) slot-levels per block: max bucket count in that block
    # across all cores
    S = []
    for blk in range(NBLK):
        m = 0
        for k in range(NCORES):
            m = max(m, int(cnt[unrank[k, blk * 128:(blk + 1) * 128]].max()))
        S.append(m)
    S = tuple(S)

    # per-pair placement: sort pairs by bucket to get slot ids
    order = np.argsort(bucket, kind="stable")
    b_sorted = bucket[order]
    slot = np.arange(len(b_sorted)) - np.searchsorted(b_sorted, b_sorted)
    cumS = np.concatenate([[0], np.cumsum(S)])
    r = rank[b_sorted]
    part = r % 128                       # table partition
    grp = cumS[r // 128] + slot          # table column group (block, slot)
    core = b_sorted // DPC
    return order, core, part, grp, pos[order], S, unrank


def kernel(bottom1, bottom2, rand_h1, rand_s1, rand_h2, rand_s2):
    _last_runs.clear()
    out_dtype = bottom1.dtype

    # ---- host: layout only (transpose / shard / pixel-half split) ----
    # [B, C, H, W] -> [b, pixel, c] -> per-core [98, BPC, 2, C]
    x = {}
    for name, t in (("x1", bottom1), ("x2", bottom2)):
        v = np.asarray(t).astype(np.float32)
        v = v.transpose(0, 2, 3, 1).reshape(B, HW, C)       # [b, p, c]
        v = v.reshape(B, 2, KH, C).transpose(2, 0, 1, 3)    # [98, b, 2, c]
        x[name] = np.ascontiguousarray(v)

    order, core, part, grp, sgn, S, unrank = _plan_tables(
        np.asarray(rand_h1), np.asarray(rand_s1),
        np.asarray(rand_h2), np.asarray(rand_s2))

    # ---- phase 1: gram matrices ----
    if "p1" not in _cache:
        _cache["p1"] = _build_phase1()
    in_maps1 = [{"x1": np.ascontiguousarray(x["x1"][:, k * BPC:(k + 1) * BPC]),
                 "x2": np.ascontiguousarray(x["x2"][:, k * BPC:(k + 1) * BPC])}
                for k in range(NCORES)]
    res1 = _run(_cache["p1"], in_maps1)
    # g [128, BPC, 4, C] -> G [b_global, c1, c2]
    g_all = np.concatenate(
        [r["g"].transpose(1, 2, 0, 3).reshape(BPC, C, C) for r in res1],
        axis=0)

    # ---- host: reshard pairs into rank-major packed tables ----
    g_pairs = g_all.reshape(B, C * C)
    vals = np.ascontiguousarray(g_pairs[:, order].T)       # [pairs, B]
    # Fold the compile-time sketch signs in as a sign-bit flip (the +-1 is
    # part of the count-sketch hash, not the data; no FLOPs involved).
    if vals.dtype.itemsize == 2:
        vals.view(np.uint16)[~sgn] ^= np.uint16(0x8000)
    else:
        vals.view(np.uint32)[~sgn] ^= np.uint32(0x80000000)

    tot = sum(S)
    tabs = []
    for k in range(NCORES):
        sel = core == k
        tk = np.zeros((128, tot, B), vals.dtype)
        tk[part[sel], grp[sel], :] = vals[sel]
        tabs.append(tk)
    ident = np.eye(128, dtype=BF16_NP)

    # ---- phase 2: segmented sums on the tensor engine ----
    key = ("p2", S)
    if key not in _cache:
        _cache[key] = _build_phase2(S)
    in_maps2 = [{"t": tabs[k], "ident": ident} for k in range(NCORES)]
    res2 = _run(_cache[key], in_maps2)

    # per-core out [128, NBLK, B]: O[bucket unrank[k, 128*blk+p], b]
    out = np.empty((D, B), np.float32)
    for k in range(NCORES):
        ok = res2[k]["out"]              # [128, NBLK, B]
        out[unrank[k]] = ok.transpose(1, 0, 2).reshape(DPC, B)
    return np.ascontiguousarray(out.T).astype(out_dtype)
